# revision 1
# baseline (speedup 1.0000x reference)
"""GATv2Conv Trainium2 kernel (8-core SPMD, full-I/O contract).

kernel(**inputs) takes the FULL inputs and returns the FULL [100000, 64] f32
output. Internally:
  - host: shard edges by destination range (core k owns dst in
    [k*12500, (k+1)*12500)); group each core's edges by 128-node dst
    "window"; lay edges on a [128, cols] slot grid (one 128-edge column per
    matmul block), padded with null edges (weight 0, one-hot-miss dst).
  - device (same program on all 8 cores, different data):
      phase1: tab[n, 72] = [h(64) | s_src(4) | s_dst(4)] for all nodes
              (h = x @ W.T; s_* folded into the matmul weights).
      phase2: per column: indirect-DMA gather tab rows by src (h + s_src)
              and s_dst by dst (element_offset); p = exp(lrelu(s)*w);
              per window: one-hot matmul scatters [p*h | p] into
              PSUM[128 nodes, 68]; out = num/(den + 1e-8).
"""
import math
import time
from contextlib import ExitStack
from dataclasses import dataclass

import numpy as np

import concourse.bass as bass
import concourse.bacc as bacc
import concourse.mybir as mybir
import concourse.tile as tile
from concourse import bass_utils

F32 = mybir.dt.float32
I32 = mybir.dt.int32

N_NODES = 100000
N_EDGES = 1600000
HEADS = 4
HEAD_DIM = 16
EPS = 1e-8
NEG = 0.2
IN_CH = 128
TABW = 72  # h(64) | s_src(4) | s_dst(4)

LAST_EXEC_NS = None
LAST_NC = None
LAST_IN_MAPS = None


@dataclass
class Cfg:
    n_nodes: int = N_NODES
    n_edges: int = N_EDGES
    cores: int = 8
    chw: int = 6
    xch: int = 4096

    @property
    def npc(self):
        return self.n_nodes // self.cores

    @property
    def wins(self):
        return math.ceil(self.npc / 128)

    @property
    def wins_pad(self):
        return math.ceil(self.wins / self.chw) * self.chw

    @property
    def nchunk(self):
        return self.wins_pad // self.chw

    @property
    def np_pad(self):
        return math.ceil(self.n_nodes / 128) * 128


def _bcast_dim(ap_obj, insert_at, count):
    newap = [list(x) for x in ap_obj.ap]
    newap.insert(insert_at, [0, count])
    return bass.AP(ap_obj.tensor, ap_obj.offset, newap)


def _make_ap(base_ap, rel_offset, dims):
    return bass.AP(base_ap.tensor, base_ap.offset + rel_offset,
                   [list(d) for d in dims])


def _host_prep(C, x, edge_index, edge_weight, W, a):
    src = np.asarray(edge_index[0], dtype=np.int64)
    dst = np.asarray(edge_index[1], dtype=np.int64)
    w = np.asarray(edge_weight, dtype=np.float32)
    E = C.n_edges

    core = dst // C.npc
    loc = dst - core * C.npc
    win = loc >> 7
    dst_in_win = (loc & 127).astype(np.float32)

    group = core * C.wins_pad + win
    order = np.argsort(group, kind="stable")
    g_sorted = group[order]
    ngroups = C.cores * C.wins_pad
    counts = np.bincount(g_sorted, minlength=ngroups)
    B = int(math.ceil(counts.max() / 128.0))
    Ktot = C.wins_pad * B

    starts = np.zeros(ngroups, dtype=np.int64)
    np.cumsum(counts[:-1], out=starts[1:])
    iw = np.arange(E, dtype=np.int64) - starts[g_sorted]

    cores_s = g_sorted // C.wins_pad
    win_s = g_sorted % C.wins_pad
    rows = iw & 127
    cols = win_s * B + (iw >> 7)

    sh = (C.cores, 128, Ktot)
    idx1 = np.zeros(sh, dtype=np.int32)
    dstc = np.full(sh, -1.0, dtype=np.float32)
    wc = np.zeros(sh, dtype=np.float32)
    idx1[cores_s, rows, cols] = src[order].astype(np.int32)
    dstc[cores_s, rows, cols] = dst_in_win[order]
    wc[cores_s, rows, cols] = w[order]

    xT = np.zeros((IN_CH, C.np_pad), dtype=np.float32)
    xT[:, :C.n_nodes] = np.asarray(x, dtype=np.float32).T

    Wt = np.ascontiguousarray(np.asarray(W, dtype=np.float32).T)  # [128, 64]
    a_np = np.asarray(a, dtype=np.float32)
    a_src = a_np[0, :, :HEAD_DIM]
    a_dst = a_np[0, :, HEAD_DIM:]
    A_src = (Wt.reshape(IN_CH, HEADS, HEAD_DIM) * a_src[None]).sum(-1)
    A_dst = (Wt.reshape(IN_CH, HEADS, HEAD_DIM) * a_dst[None]).sum(-1)
    rhs_ext = np.ascontiguousarray(
        np.concatenate([Wt, A_src, A_dst], axis=1), dtype=np.float32)
    iota = np.ascontiguousarray(
        np.broadcast_to(np.arange(128, dtype=np.float32), (128, 128)))
    ident = np.eye(128, dtype=np.float32)
    idx3 = np.zeros((C.cores, 128, C.wins_pad), dtype=np.int32)
    for c in range(C.cores):
        base = c * C.npc
        for wv in range(C.wins_pad):
            n0 = base + wv * 128
            ids = np.arange(n0, n0 + 128)
            # clamp to valid local range (pad windows/tail read real rows)
            ids = np.minimum(ids, base + C.npc - 1)
            idx3[c, :, wv] = ids

    in_maps = []
    for c in range(C.cores):
        in_maps.append(dict(
            xT=xT, rhs_ext=rhs_ext, iota=iota, ident=ident, idx3=idx3[c],
            idx1=idx1[c], dstc=dstc[c], wc=wc[c]))
    return in_maps, B


def _build_program(C, B, num_devices=None):
    Kc = C.chw * B
    ND = num_devices or C.cores
    TAB = C.np_pad

    nc = bacc.Bacc("TRN2", target_bir_lowering=False, debug=False,
                   enable_asserts=False, num_devices=ND)
    xT_d = nc.dram_tensor("xT", [IN_CH, TAB], F32, kind="ExternalInput")
    re_d = nc.dram_tensor("rhs_ext", [IN_CH, TABW], F32, kind="ExternalInput")
    io_d = nc.dram_tensor("iota", [128, 128], F32, kind="ExternalInput")
    id_d = nc.dram_tensor("ident", [128, 128], F32, kind="ExternalInput")
    idx1_d = nc.dram_tensor("idx1", [128, C.wins_pad * B], I32,
                            kind="ExternalInput")
    idx3_d = nc.dram_tensor("idx3", [128, C.wins_pad], I32,
                            kind="ExternalInput")
    dstc_d = nc.dram_tensor("dstc", [128, C.wins_pad * B], F32,
                            kind="ExternalInput")
    wc_d = nc.dram_tensor("wc", [128, C.wins_pad * B], F32,
                          kind="ExternalInput")
    tab_d = nc.dram_tensor("tab", [TAB, TABW], F32, kind="Internal")
    out_d = nc.dram_tensor("out", [C.wins_pad * 128, 64], F32,
                           kind="ExternalOutput")

    with tile.TileContext(nc) as tc, ExitStack() as ctx:
        const = ctx.enter_context(tc.tile_pool(name="const", bufs=1))
        iota_t = const.tile([128, 128], F32)
        nc.sync.dma_start(out=iota_t[:], in_=io_d[:])
        re_t = const.tile([128, TABW], F32)
        nc.sync.dma_start(out=re_t[:], in_=re_d[:])
        id_t = const.tile([128, 128], F32)
        nc.sync.dma_start(out=id_t[:], in_=id_d[:])

        # ---------------- phase 1: tab = [h | s_src | s_dst] ----------------
        with tc.tile_pool(name="xload", bufs=2) as xp, \
             tc.tile_pool(name="hstage", bufs=3) as hp, \
             tc.tile_pool(name="psh", bufs=4, space="PSUM") as php:
            n_done = 0
            while n_done < TAB:
                csz = min(C.xch, TAB - n_done)
                xt_t = xp.tile([128, C.xch], F32, tag="xt")
                nc.sync.dma_start(out=xt_t[:, :csz],
                                  in_=xT_d[:, n_done:n_done + csz])
                ntile = csz // 128
                GRP = 8
                for j0 in range(0, ntile, GRP):
                    grp = min(GRP, ntile - j0)
                    hs_t = hp.tile([128, GRP, TABW], F32, tag="hs")
                    for j in range(j0, j0 + grp):
                        ph = php.tile([128, TABW], F32, tag="ph")
                        nc.tensor.matmul(
                            out=ph[:], lhsT=xt_t[:, j * 128:(j + 1) * 128],
                            rhs=re_t[:], start=True, stop=True)
                        nc.vector.tensor_copy(out=hs_t[:, j - j0, :],
                                              in_=ph[:])
                    r0 = n_done + j0 * 128
                    dst_ap = _make_ap(
                        tab_d[:], r0 * TABW,
                        [[TABW, 128], [128 * TABW, grp], [1, TABW]])
                    nc.sync.dma_start(out=dst_ap, in_=hs_t[:, :grp, :])
                n_done += csz

        # ---------------- phase 2: edges ----------------
        sb = ctx.enter_context(tc.tile_pool(name="edge", bufs=2))
        wb = ctx.enter_context(tc.tile_pool(name="winb", bufs=2))
        psw = ctx.enter_context(tc.tile_pool(name="psw", bufs=2, space="PSUM"))

        for c in range(C.nchunk):
            k0 = c * Kc
            idx1_t = sb.tile([128, Kc], I32, tag="idx1")
            idx3_t = sb.tile([128, C.chw], I32, tag="idx3")
            dstc_t = sb.tile([128, Kc], F32, tag="dstc")
            wc_t = sb.tile([128, Kc], F32, tag="wc")
            nc.sync.dma_start(out=idx1_t[:], in_=idx1_d[:, k0:k0 + Kc])
            nc.sync.dma_start(out=idx3_t[:],
                              in_=idx3_d[:, c * C.chw:(c + 1) * C.chw])
            nc.sync.dma_start(out=dstc_t[:], in_=dstc_d[:, k0:k0 + Kc])
            nc.sync.dma_start(out=wc_t[:], in_=wc_d[:, k0:k0 + Kc])

            g = sb.tile([128, Kc, TABW], F32, tag="g")
            for k in range(Kc):
                nc.gpsimd.indirect_dma_start(
                    out=g[:, k, :], out_offset=None, in_=tab_d[:],
                    in_offset=bass.IndirectOffsetOnAxis(
                        ap=idx1_t[:, k:k + 1], axis=0))

            ot = wb.tile([128, C.chw, 64], F32, tag="ot")
            for w in range(C.chw):
                b0 = w * B
                oh = wb.tile([128, B, 128], F32, tag="oh")
                nc.vector.tensor_tensor(
                    out=oh[:], in0=_bcast_dim(iota_t[:], 1, B),
                    in1=dstc_t[:, b0:b0 + B].to_broadcast([128, B, 128]),
                    op=mybir.AluOpType.is_equal)

                # s_dst for this window's 128 nodes, expand to edges via
                # PE-transposed one-hot
                sdw = wb.tile([128, 4], F32, tag="sdw")
                nc.gpsimd.indirect_dma_start(
                    out=sdw[:], out_offset=None, in_=tab_d[:],
                    in_offset=bass.IndirectOffsetOnAxis(
                        ap=idx3_t[:, w:w + 1], axis=0),
                    element_offset=68)
                sde_ps = psw.tile([128, B, 4], F32, tag="sde")
                for j in range(B):
                    ohT_ps = psw.tile([128, 128], F32, tag="ohT")
                    nc.tensor.transpose(out=ohT_ps[:], in_=oh[:, j, :],
                                        identity=id_t[:])
                    ohT_sb = wb.tile([128, 128], F32, tag="ohTs")
                    nc.vector.tensor_copy(out=ohT_sb[:], in_=ohT_ps[:])
                    nc.tensor.matmul(out=sde_ps[:, j, :], lhsT=ohT_sb[:],
                                     rhs=sdw[:], start=True, stop=True)

                logit = wb.tile([128, B, 4], F32, tag="logit")
                nc.vector.tensor_add(out=logit[:], in0=g[:, b0:b0 + B, 64:68],
                                     in1=sde_ps[:])
                nc.vector.scalar_tensor_tensor(
                    out=logit[:], in0=logit[:], scalar=NEG, in1=logit[:],
                    op0=mybir.AluOpType.mult, op1=mybir.AluOpType.max)
                nc.vector.tensor_mul(
                    out=logit[:], in0=logit[:],
                    in1=wc_t[:, b0:b0 + B].to_broadcast([128, B, 4]))
                p = wb.tile([128, B, 4], F32, tag="p")
                nc.scalar.activation(p[:], logit[:],
                                     mybir.ActivationFunctionType.Exp)

                pay = wb.tile([128, B, 68], F32, tag="pay")
                pv = p[:].to_broadcast([128, B, 4, 16])
                gv = g[:, b0:b0 + B, 0:64].rearrange(
                    "p k (h d) -> p k h d", d=16)
                ov = pay[:, :, 0:64].rearrange("p k (h d) -> p k h d", d=16)
                nc.vector.tensor_mul(out=ov, in0=gv, in1=pv)
                nc.vector.tensor_copy(out=pay[:, :, 64:68], in_=p[:])

                acc = psw.tile([128, 68], F32, tag="acc")
                for j in range(B):
                    nc.tensor.matmul(
                        out=acc[:], lhsT=oh[:, j, :], rhs=pay[:, j, :],
                        start=(j == 0), stop=(j == B - 1))

                den = wb.tile([128, 4], F32, tag="den")
                nc.vector.tensor_scalar_add(out=den[:], in0=acc[:, 64:68],
                                            scalar1=EPS)
                rec = wb.tile([128, 4], F32, tag="rec")
                nc.vector.reciprocal(out=rec[:], in_=den[:])
                nc.vector.tensor_mul(
                    out=ot[:, w, :].rearrange("p (h d) -> p h d", d=16),
                    in0=acc[:, 0:64].rearrange("p (h d) -> p h d", d=16),
                    in1=rec[:].to_broadcast([128, 4, 16]))
            r0 = c * C.chw * 128
            dst_ap = _make_ap(out_d[:], r0 * 64,
                              [[64, 128], [128 * 64, C.chw], [1, 64]])
            nc.sync.dma_start(out=dst_ap, in_=ot[:])

    nc.compile()
    return nc


def kernel(x, edge_index, edge_weight, W, a):
    global LAST_EXEC_NS
    C = Cfg()
    t0 = time.time()
    in_maps, B = _host_prep(C, x, edge_index, edge_weight, W, a)
    t1 = time.time()
    nc = _build_program(C, B)
    global LAST_NC, LAST_IN_MAPS
    LAST_NC = nc
    LAST_IN_MAPS = in_maps
    t2 = time.time()
    res = bass_utils.run_bass_kernel_spmd(
        nc, in_maps, core_ids=list(range(C.cores)))
    t3 = time.time()
    print(f"[kernel] host_prep {t1-t0:.1f}s  build+compile {t2-t1:.1f}s  "
          f"exec(all-in) {t3-t2:.1f}s  B={B}")
    LAST_EXEC_NS = res.exec_time_ns
    parts = [res.results[c]["out"][:C.npc] for c in range(C.cores)]
    return np.ascontiguousarray(np.concatenate(parts, axis=0))



# revision 5
# speedup vs baseline: 1580.2565x; 1580.2565x over previous
"""GATv2Conv Trainium2 kernel (8-core SPMD, full-I/O contract) — v2.

kernel(**inputs) takes the FULL inputs and returns the FULL [100000, 64] f32
output.

Design:
  - Edges sharded by destination: core k owns dst in [k*12500, (k+1)*12500).
  - Per-core node permutation pi_k puts the core's own 12500 nodes first, so
    per-window s_dst slices live at static offsets (kept in SBUF, no gather).
  - Phase 1 (device): tab[row] = [h bf16(64) | s_src bf16(4) | s_dst bf16(4) |
    pad] as 128-bf16 (256B) rows in HBM, row = pi_k(node); s_dst for the
    core's own 12544 rows is also kept in SBUF (sdall, f32).
  - Phase 2 (device), per 128-dst-node window:
      * dma_gather (gpsimd/SWDGE, int16 idx wrapped in 16 partitions) pulls
        the 256B tab rows for the window's edges. Indices are pi-rows split
        into 4 classes of <=32768 rows so int16 reaches the whole table;
        edges are grouped by class into separate column blocks of the
        [128, Btot] slot grid (one 128-edge column per scatter matmul).
      * one-hot scatter: oh[e,n] = (dstc[e]==n) via DVE is_equal; partial
        sums accumulate in PSUM via bf16 matmuls.
      * s_dst per edge via ohT built from a stride-0 (partition-broadcast)
        DMA of the transposed slot->dst map + is_equal; per-column matmul
        against the window's s_dst.
      * p = exp(leaky_relu(ss+sd)*w); out = (sum p*h)/(sum p + eps).

_build_program(repeat=R) repeats phase1+phase2 R times in one NEFF; used by
test.py to time exec as (wall_R - wall_1)/(R-1), cancelling fixed overheads.
"""
import math
import time
from contextlib import ExitStack

import numpy as np
import ml_dtypes

import concourse.bass as bass
import concourse.bacc as bacc
import concourse.mybir as mybir
import concourse.tile as tile
from concourse import bass_utils

F32 = mybir.dt.float32
BF16 = mybir.dt.bfloat16
I16 = mybir.dt.int16

N_NODES = 100000
N_EDGES = 1600000
HEADS = 4
HEAD_DIM = 16
EPS = 1e-8
NEG = 0.2
IN_CH = 128

CORES = 8
NPC = N_NODES // CORES            # 12500
WINS = math.ceil(NPC / 128)       # 98
OWN_PAD = WINS * 128              # 12544
NP_PAD = math.ceil(N_NODES / 128) * 128   # 100096
NCLASS = math.ceil(NP_PAD / 32768)        # 4
CLS_ROWS = [min(32768, NP_PAD - r * 32768) for r in range(NCLASS)]
XCH = 4096
OG = 7                            # windows per output DMA (98 = 14*7)

LAST_EXEC_NS = None
LAST_NC = None
LAST_IN_MAPS = None
LAST_BR = None


def _make_ap(base_ap, rel_offset, dims):
    return bass.AP(base_ap.tensor, base_ap.offset + rel_offset,
                   [list(d) for d in dims])


def _bcast_dim(ap_obj, insert_at, count):
    newap = [list(x) for x in ap_obj.ap]
    newap.insert(insert_at, [0, count])
    return bass.AP(ap_obj.tensor, ap_obj.offset, newap)


def _host_prep(x, edge_index, edge_weight, W, a):
    src = np.asarray(edge_index[0], dtype=np.int64)
    dst = np.asarray(edge_index[1], dtype=np.int64)
    ew = np.asarray(edge_weight, dtype=np.float32)

    xT = np.asarray(x, dtype=np.float32).T          # [128, N]
    W32 = np.asarray(W, dtype=np.float32)
    a32 = np.asarray(a, dtype=np.float32)
    Wt = W32.T                                       # [128, 64]
    a_src = a32[0, :, :HEAD_DIM]
    a_dst = a32[0, :, HEAD_DIM:]
    A_src = (Wt.reshape(IN_CH, HEADS, HEAD_DIM) * a_src[None]).sum(-1)
    A_dst = (Wt.reshape(IN_CH, HEADS, HEAD_DIM) * a_dst[None]).sum(-1)
    rhs_ext = np.ascontiguousarray(
        np.concatenate([Wt, A_src, A_dst], axis=1)).astype(ml_dtypes.bfloat16)
    iota = np.ascontiguousarray(
        np.broadcast_to(np.arange(128, dtype=np.float32), (128, 128))
    ).astype(ml_dtypes.bfloat16)
    iop = np.arange(128, dtype=np.float32).reshape(128, 1).astype(
        ml_dtypes.bfloat16)

    core = dst // NPC
    loc = dst - core * NPC
    win = loc >> 7
    diw = (loc & 127).astype(np.float32)

    percore = []
    for k in range(CORES):
        m = core == k
        s = src[m]
        base = k * NPC
        own = (s >= base) & (s < base + NPC)
        row = np.where(own, s - base,
                       NPC + np.where(s < base, s, s - NPC))
        cls = (row >> 15).astype(np.int64)
        wv = win[m]
        key = wv * NCLASS + cls
        order = np.lexsort((row, key))
        percore.append(dict(row=row[order], cls=cls[order], key=key[order],
                            diw=diw[m][order], ew=ew[m][order]))

    # class column counts (shared across cores -> same program)
    Br = np.zeros(NCLASS, dtype=np.int64)
    for k in range(CORES):
        pc = percore[k]
        counts = np.bincount(pc["key"], minlength=WINS * NCLASS)
        per_cls = counts.reshape(WINS, NCLASS).max(axis=0)
        Br = np.maximum(Br, (per_cls + 127) // 128)
    Br = [int(b) for b in Br]
    coff = np.concatenate([[0], np.cumsum(Br)])
    Btot = int(coff[-1])

    glob = np.arange(N_NODES, dtype=np.int64)
    in_maps = []
    for k in range(CORES):
        pc = percore[k]
        base = k * NPC
        counts = np.bincount(pc["key"], minlength=WINS * NCLASS)
        starts = np.zeros(WINS * NCLASS, dtype=np.int64)
        np.cumsum(counts[:-1], out=starts[1:])
        iw = np.arange(len(pc["row"]), dtype=np.int64) - starts[pc["key"]]
        wv = pc["key"] // NCLASS
        cls = pc["cls"]
        col = coff[cls] + (iw >> 7)
        p = iw & 127

        idx16 = np.zeros((16, WINS * Btot * 8), dtype=np.int16)
        icol = wv * (Btot * 8) + (coff[cls] + (iw >> 7)) * 8 + ((iw & 127) >> 4)
        irow = iw & 15
        idx16[irow, icol] = (pc["row"] - (cls << 15)).astype(np.int16)
        idx16 = np.tile(idx16, (8, 1))

        dstc = np.full((128, WINS * Btot), -1.0, dtype=np.float32)
        wc = np.zeros((128, WINS * Btot), dtype=np.float32)
        scol = wv * Btot + col
        dstc[p, scol] = pc["diw"]
        wc[p, scol] = pc["ew"]
        dstf = np.ascontiguousarray(dstc.T.reshape(1, -1)).astype(
            ml_dtypes.bfloat16)
        dstc_bf = dstc.astype(ml_dtypes.bfloat16)

        order_nodes = np.concatenate(
            [glob[base:base + NPC], glob[:base], glob[base + NPC:]])
        xTp = np.zeros((IN_CH, NP_PAD), dtype=ml_dtypes.bfloat16)
        xTp[:, :N_NODES] = xT[:, order_nodes].astype(ml_dtypes.bfloat16)

        in_maps.append(dict(
            xT=xTp, rhs_ext=rhs_ext, iota=iota, iop=iop,
            idx16=idx16, dstc=dstc_bf, dstf=dstf, wc=wc))
    return in_maps, Br


def _phase1(nc, tc, xT_d, re_t, tab_d, sdall):
    with tc.tile_pool(name="xload", bufs=2) as xp, \
         tc.tile_pool(name="hstage", bufs=3) as hp, \
         tc.tile_pool(name="psh", bufs=4, space="PSUM") as php:
        n_done = 0
        while n_done < NP_PAD:
            csz = min(XCH, NP_PAD - n_done)
            xt_t = xp.tile([128, XCH], BF16, tag="xt")
            nc.sync.dma_start(out=xt_t[:, :csz],
                              in_=xT_d[:, n_done:n_done + csz])
            ntile = csz // 128
            GRP = 8
            for j0 in range(0, ntile, GRP):
                grp = min(GRP, ntile - j0)
                hs_t = hp.tile([128, GRP, 128], BF16, tag="hs")
                nc.vector.memset(hs_t[:, :, 72:128], 0.0)
                for j in range(j0, j0 + grp):
                    bi = (n_done + j * 128) // 128
                    ph = php.tile([128, 72], F32, tag="ph")
                    nc.tensor.matmul(
                        out=ph[:], lhsT=xt_t[:, j * 128:(j + 1) * 128],
                        rhs=re_t[:], start=True, stop=True)
                    nc.vector.tensor_copy(out=hs_t[:, j - j0, 0:72],
                                          in_=ph[:])
                    if bi < WINS:
                        nc.vector.tensor_copy(out=sdall[:, bi, :],
                                              in_=ph[:, 68:72])
                r0 = n_done + j0 * 128
                dst_ap = _make_ap(
                    tab_d[:], r0 * 128,
                    [[128, 128], [128 * 128, grp], [1, 128]])
                nc.sync.dma_start(out=dst_ap, in_=hs_t[:, :grp, :])
            n_done += csz


def _phase2(nc, tc, pools, Br, coff, Btot, iota_t, iop_t, sdall,
            idx_d, dstc_d, dstf_d, wc_d, tab_d, out_d):
    ld, gp, cw, op, pss, psa = pools
    ot = None
    for w in range(WINS):
        idx_t = ld.tile([128, Btot * 8], I16, tag="idx")
        nc.sync.dma_start(out=idx_t[:],
                          in_=idx_d[:, w * Btot * 8:(w + 1) * Btot * 8])
        dstc_t = ld.tile([128, Btot], BF16, tag="dstc")
        nc.sync.dma_start(out=dstc_t[:],
                          in_=dstc_d[:, w * Btot:(w + 1) * Btot])
        wc_t = ld.tile([128, Btot], F32, tag="wc")
        nc.sync.dma_start(out=wc_t[:],
                          in_=wc_d[:, w * Btot:(w + 1) * Btot])
        # partition-broadcast of dstf slice (stride-0 DRAM src)
        bc_t = ld.tile([128, Btot * 128], BF16, tag="bc")
        src_ap = bass.AP(dstf_d[:].tensor, w * Btot * 128,
                         [[0, 128], [1, Btot * 128]])
        nc.sync.dma_start(out=bc_t[:], in_=src_ap)

        g = gp.tile([128, Btot, 128], BF16, tag="g")
        for r in range(NCLASS):
            if Br[r] == 0:
                continue
            view = tab_d[r * 32768: r * 32768 + CLS_ROWS[r]]
            nc.gpsimd.dma_gather(
                g[:, coff[r]:coff[r + 1], :], view,
                idx_t[:, coff[r] * 8:coff[r + 1] * 8],
                Br[r] * 128, Br[r] * 128, 128)

        oh = cw.tile([128, Btot, 128], BF16, tag="oh")
        nc.vector.tensor_tensor(
            out=oh[:], in0=_bcast_dim(iota_t[:], 1, Btot),
            in1=dstc_t[:].to_broadcast([128, Btot, 128]),
            op=mybir.AluOpType.is_equal)
        ohT = cw.tile([128, Btot * 128], BF16, tag="ohT")
        nc.vector.tensor_tensor(
            out=ohT[:], in0=bc_t[:],
            in1=iop_t[:].to_broadcast([128, Btot * 128]),
            op=mybir.AluOpType.is_equal)

        sdwb = cw.tile([128, 4], BF16, tag="sdwb")
        nc.vector.tensor_copy(out=sdwb[:], in_=sdall[:, w, :])
        sde_ps = pss.tile([128, Btot * 4], F32, tag="sde")
        for c in range(Btot):
            nc.tensor.matmul(out=sde_ps[:, c * 4:(c + 1) * 4],
                             lhsT=ohT[:, c * 128:(c + 1) * 128],
                             rhs=sdwb[:], start=True, stop=True)

        logit = cw.tile([128, Btot, 4], F32, tag="logit")
        nc.vector.tensor_add(
            out=logit[:], in0=g[:, :, 64:68],
            in1=sde_ps[:].rearrange("p (a b) -> p a b", b=4))
        nc.vector.scalar_tensor_tensor(
            out=logit[:], in0=logit[:], scalar=NEG, in1=logit[:],
            op0=mybir.AluOpType.mult, op1=mybir.AluOpType.max)
        nc.vector.tensor_mul(
            out=logit[:], in0=logit[:],
            in1=wc_t[:].to_broadcast([128, Btot, 4]))
        p = cw.tile([128, Btot, 4], F32, tag="p")
        nc.scalar.activation(p[:], logit[:],
                             mybir.ActivationFunctionType.Exp)

        pay = cw.tile([128, Btot, 68], BF16, tag="pay")
        pv = p[:].to_broadcast([128, Btot, 4, 16])
        gv = g[:, :, 0:64].rearrange("p k (h d) -> p k h d", d=16)
        ov = pay[:, :, 0:64].rearrange("p k (h d) -> p k h d", d=16)
        nc.vector.tensor_mul(out=ov, in0=gv, in1=pv)
        nc.vector.tensor_copy(out=pay[:, :, 64:68], in_=p[:])

        acc = psa.tile([128, 68], F32, tag="acc")
        for c in range(Btot):
            nc.tensor.matmul(
                out=acc[:], lhsT=oh[:, c, :], rhs=pay[:, c, :],
                start=(c == 0), stop=(c == Btot - 1))

        if w % OG == 0:
            ot = op.tile([128, OG, 64], F32, tag="ot")
        den = cw.tile([128, 4], F32, tag="den")
        nc.vector.tensor_scalar_add(out=den[:], in0=acc[:, 64:68],
                                    scalar1=EPS)
        rec = cw.tile([128, 4], F32, tag="rec")
        nc.vector.reciprocal(out=rec[:], in_=den[:])
        nc.vector.tensor_mul(
            out=ot[:, w % OG, :].rearrange("p (h d) -> p h d", d=16),
            in0=acc[:, 0:64].rearrange("p (h d) -> p h d", d=16),
            in1=rec[:].to_broadcast([128, 4, 16]))
        if w % OG == OG - 1:
            r0 = (w - OG + 1) * 128
            dst_ap = _make_ap(out_d[:], r0 * 64,
                              [[64, 128], [128 * 64, OG], [1, 64]])
            nc.sync.dma_start(out=dst_ap, in_=ot[:])


def _build_program(Br, num_devices=CORES, repeat=1):
    coff = [0]
    for b in Br:
        coff.append(coff[-1] + b)
    Btot = coff[-1]

    nc = bacc.Bacc("TRN2", target_bir_lowering=False, debug=False,
                   enable_asserts=False, num_devices=num_devices)
    xT_d = nc.dram_tensor("xT", [IN_CH, NP_PAD], BF16, kind="ExternalInput")
    re_d = nc.dram_tensor("rhs_ext", [IN_CH, 72], BF16, kind="ExternalInput")
    io_d = nc.dram_tensor("iota", [128, 128], BF16, kind="ExternalInput")
    iop_d = nc.dram_tensor("iop", [128, 1], BF16, kind="ExternalInput")
    idx_d = nc.dram_tensor("idx16", [128, WINS * Btot * 8], I16,
                           kind="ExternalInput")
    dstc_d = nc.dram_tensor("dstc", [128, WINS * Btot], BF16,
                            kind="ExternalInput")
    dstf_d = nc.dram_tensor("dstf", [1, WINS * Btot * 128], BF16,
                            kind="ExternalInput")
    wc_d = nc.dram_tensor("wc", [128, WINS * Btot], F32,
                          kind="ExternalInput")
    tab_d = nc.dram_tensor("tab", [NP_PAD, 128], BF16, kind="Internal")
    out_d = nc.dram_tensor("out", [OWN_PAD, 64], F32, kind="ExternalOutput")

    with tile.TileContext(nc) as tc, ExitStack() as ctx:
        const = ctx.enter_context(tc.tile_pool(name="const", bufs=1))
        iota_t = const.tile([128, 128], BF16)
        nc.sync.dma_start(out=iota_t[:], in_=io_d[:])
        iop_t = const.tile([128, 1], BF16)
        nc.sync.dma_start(out=iop_t[:], in_=iop_d[:])
        re_t = const.tile([128, 72], BF16)
        nc.sync.dma_start(out=re_t[:], in_=re_d[:])
        sdp = ctx.enter_context(tc.tile_pool(name="sdp", bufs=1))

        ld = ctx.enter_context(tc.tile_pool(name="ld", bufs=3))
        gp = ctx.enter_context(tc.tile_pool(name="gp", bufs=2))
        cw = ctx.enter_context(tc.tile_pool(name="cw", bufs=2))
        op = ctx.enter_context(tc.tile_pool(name="op", bufs=2))
        pss = ctx.enter_context(tc.tile_pool(name="pss", bufs=2, space="PSUM"))
        psa = ctx.enter_context(tc.tile_pool(name="psa", bufs=2, space="PSUM"))
        pools = (ld, gp, cw, op, pss, psa)

        for _rep in range(repeat):
            sdall = sdp.tile([128, WINS, 4], F32, tag="sdall")
            _phase1(nc, tc, xT_d, re_t, tab_d, sdall)
            _phase2(nc, tc, pools, Br, coff, Btot, iota_t, iop_t, sdall,
                    idx_d, dstc_d, dstf_d, wc_d, tab_d, out_d)

    nc.compile()
    return nc


def kernel(x, edge_index, edge_weight, W, a):
    global LAST_EXEC_NS, LAST_NC, LAST_IN_MAPS, LAST_BR
    t0 = time.time()
    in_maps, Br = _host_prep(x, edge_index, edge_weight, W, a)
    t1 = time.time()
    nc = _build_program(Br)
    LAST_NC = nc
    LAST_IN_MAPS = in_maps
    LAST_BR = Br
    t2 = time.time()
    res = bass_utils.run_bass_kernel_spmd(
        nc, in_maps, core_ids=list(range(CORES)))
    t3 = time.time()
    print(f"[kernel] host_prep {t1-t0:.1f}s  build+compile {t2-t1:.1f}s  "
          f"exec(all-in) {t3-t2:.1f}s  Br={Br}")
    LAST_EXEC_NS = res.exec_time_ns
    parts = [res.results[c]["out"][:NPC] for c in range(CORES)]
    return np.ascontiguousarray(np.concatenate(parts, axis=0))


# revision 20
# speedup vs baseline: 2397.2967x; 1.5170x over previous
"""GATv2Conv Trainium2 kernel (8-core SPMD, full-I/O contract) — v2.

kernel(**inputs) takes the FULL inputs and returns the FULL [100000, 64] f32
output.

Design:
  - Edges sharded by destination: core k owns dst in [k*12500, (k+1)*12500).
  - Per-core node permutation pi_k puts the core's own 12500 nodes first, so
    per-window s_dst slices live at static offsets (kept in SBUF, no gather).
  - Phase 1 (device): tab[row] = [h bf16(64) | s_src bf16(4) | s_dst bf16(4) |
    pad] as 128-bf16 (256B) rows in HBM, row = pi_k(node); s_dst for the
    core's own 12544 rows is also kept in SBUF (sdall, f32).
  - Phase 2 (device), per 128-dst-node window:
      * dma_gather (gpsimd/SWDGE, int16 idx wrapped in 16 partitions) pulls
        the 256B tab rows for the window's edges. Indices are pi-rows split
        into 4 classes of <=32768 rows so int16 reaches the whole table;
        edges are grouped by class into separate column blocks of the
        [128, Btot] slot grid (one 128-edge column per scatter matmul).
      * one-hot scatter: oh[e,n] = (dstc[e]==n) via DVE is_equal; partial
        sums accumulate in PSUM via bf16 matmuls.
      * s_dst per edge via ohT built from a stride-0 (partition-broadcast)
        DMA of the transposed slot->dst map + is_equal; per-column matmul
        against the window's s_dst.
      * p = exp(leaky_relu(ss+sd)*w); out = (sum p*h)/(sum p + eps).

_build_program(repeat=R) repeats phase1+phase2 R times in one NEFF; used by
test.py to time exec as (wall_R - wall_1)/(R-1), cancelling fixed overheads.
"""
import math
import time
from contextlib import ExitStack

import numpy as np
import ml_dtypes

import concourse.bass as bass
import concourse.bacc as bacc
import concourse.mybir as mybir
import concourse.tile as tile
from concourse import bass_utils

F32 = mybir.dt.float32
BF16 = mybir.dt.bfloat16
I16 = mybir.dt.int16
I8 = mybir.dt.int8

N_NODES = 100000
N_EDGES = 1600000
HEADS = 4
HEAD_DIM = 16
EPS = 1e-8
NEG = 0.2
IN_CH = 128

CORES = 8
NPC = N_NODES // CORES            # 12500
WINS = math.ceil(NPC / 128)       # 98
OWN_PAD = WINS * 128              # 12544
NP_PAD = math.ceil(N_NODES / 128) * 128   # 100096
NBLK = NP_PAD // 128                      # 782
NCLASS = math.ceil(NP_PAD / 32768)        # 4
CLS_ROWS = [min(32768, NP_PAD - r * 32768) for r in range(NCLASS)]
XCH = 4096
OG = 7                            # windows per output DMA (98 = 14*7)

LAST_EXEC_NS = None
LAST_NC = None
LAST_IN_MAPS = None
LAST_BR = None


def _make_ap(base_ap, rel_offset, dims):
    return bass.AP(base_ap.tensor, base_ap.offset + rel_offset,
                   [list(d) for d in dims])


def _bcast_dim(ap_obj, insert_at, count):
    newap = [list(x) for x in ap_obj.ap]
    newap.insert(insert_at, [0, count])
    return bass.AP(ap_obj.tensor, ap_obj.offset, newap)


def _host_prep(x, edge_index, edge_weight, W, a):
    src = np.asarray(edge_index[0], dtype=np.int64)
    dst = np.asarray(edge_index[1], dtype=np.int64)
    ew = np.asarray(edge_weight, dtype=np.float32)

    xT = np.asarray(x, dtype=np.float32).T          # [128, N]
    W32 = np.asarray(W, dtype=np.float32)
    a32 = np.asarray(a, dtype=np.float32)
    Wt = W32.T                                       # [128, 64]
    a_src = a32[0, :, :HEAD_DIM]
    a_dst = a32[0, :, HEAD_DIM:]
    A_src = (Wt.reshape(IN_CH, HEADS, HEAD_DIM) * a_src[None]).sum(-1)
    A_dst = (Wt.reshape(IN_CH, HEADS, HEAD_DIM) * a_dst[None]).sum(-1)
    rhs_ext = np.ascontiguousarray(
        np.concatenate([Wt, A_src, A_dst], axis=1)).astype(ml_dtypes.bfloat16)
    iota = np.ascontiguousarray(
        np.broadcast_to(np.arange(128, dtype=np.float32), (128, 128))
    ).astype(ml_dtypes.bfloat16)
    iop = np.arange(128, dtype=np.int8).reshape(128, 1)

    core = dst // NPC
    loc = dst - core * NPC
    win = loc >> 7
    diw = (loc & 127).astype(np.float32)

    percore = []
    for k in range(CORES):
        m = core == k
        s = src[m]
        base = k * NPC
        own = (s >= base) & (s < base + NPC)
        row = np.where(own, s - base,
                       NPC + np.where(s < base, s, s - NPC))
        # tab rows are swizzled so phase-1 writes are per-partition
        # contiguous: node at (block b, partition p) -> tab row p*NBLK + b
        row = (row & 127) * NBLK + (row >> 7)
        cls = (row >> 15).astype(np.int64)
        wv = win[m]
        key = wv * NCLASS + cls
        order = np.lexsort((row, key))
        percore.append(dict(row=row[order], cls=cls[order], key=key[order],
                            diw=diw[m][order], ew=ew[m][order]))

    # class column counts (shared across cores -> same program)
    Br = np.zeros(NCLASS, dtype=np.int64)
    cnt_max = np.zeros((WINS, NCLASS), dtype=np.int64)
    for k in range(CORES):
        pc = percore[k]
        counts = np.bincount(pc["key"], minlength=WINS * NCLASS)
        cnt_max = np.maximum(cnt_max, counts.reshape(WINS, NCLASS))
        per_cls = counts.reshape(WINS, NCLASS).max(axis=0)
        Br = np.maximum(Br, (per_cls + 127) // 128)
    Br = [int(b) for b in Br]
    coff = np.concatenate([[0], np.cumsum(Br)])
    Btot = int(coff[-1])
    # static per-(window,class) gather sizes: cross-core max, 128-aligned
    # (full columns -> every slot of a gathered column is written)
    nidx = ((cnt_max + 127) // 128 * 128).astype(np.int64)  # [WINS, NCLASS]

    glob = np.arange(N_NODES, dtype=np.int64)
    in_maps = []
    for k in range(CORES):
        pc = percore[k]
        base = k * NPC
        counts = np.bincount(pc["key"], minlength=WINS * NCLASS)
        starts = np.zeros(WINS * NCLASS, dtype=np.int64)
        np.cumsum(counts[:-1], out=starts[1:])
        iw = np.arange(len(pc["row"]), dtype=np.int64) - starts[pc["key"]]
        wv = pc["key"] // NCLASS
        cls = pc["cls"]
        col = coff[cls] + (iw >> 7)
        p = iw & 127

        idx16 = np.zeros((16, WINS * Btot * 8), dtype=np.int16)
        icol = wv * (Btot * 8) + (coff[cls] + (iw >> 7)) * 8 + ((iw & 127) >> 4)
        irow = iw & 15
        idx16[irow, icol] = (pc["row"] - (cls << 15)).astype(np.int16)
        idx16 = np.tile(idx16, (8, 1))

        dstc = np.full((128, WINS * Btot), -1.0, dtype=np.float32)
        wc = np.zeros((128, WINS * Btot), dtype=np.float32)
        scol = wv * Btot + col
        dstc[p, scol] = pc["diw"]
        wc[p, scol] = pc["ew"]
        dstf = np.ascontiguousarray(dstc.T.reshape(1, -1)).astype(np.int8)
        dstc_bf = dstc.astype(ml_dtypes.bfloat16)

        order_nodes = np.concatenate(
            [glob[base:base + NPC], glob[:base], glob[base + NPC:]])
        xTp = np.zeros((IN_CH, NP_PAD), dtype=ml_dtypes.bfloat16)
        xTp[:, :N_NODES] = xT[:, order_nodes].astype(ml_dtypes.bfloat16)

        in_maps.append(dict(
            xT=xTp, rhs_ext=rhs_ext, iota=iota, iop=iop,
            idx16=idx16, dstc=dstc_bf, dstf=dstf, wc=wc))
    return in_maps, (Br, [[int(v) for v in row] for row in nidx])


def _phase1(nc, tc, xT_d, re_t, tab_d, sdall):
    GRP = 16    # tab blocks per write DMA (per-partition 16*256B contiguous)
    SUB = 4     # blocks per PSUM tile / DVE copy
    with tc.tile_pool(name="xload", bufs=2) as xp, \
         tc.tile_pool(name="hstage", bufs=3) as hp, \
         tc.tile_pool(name="psh", bufs=3, space="PSUM") as php:
        n_done = 0
        while n_done < NP_PAD:
            csz = min(XCH, NP_PAD - n_done)
            xt_t = xp.tile([128, XCH], BF16, tag="xt")
            nc.sync.dma_start(out=xt_t[:, :csz],
                              in_=xT_d[:, n_done:n_done + csz])
            ntile = csz // 128
            for j0 in range(0, ntile, GRP):
                grp = min(GRP, ntile - j0)
                hs_t = hp.tile([128, GRP, 128], BF16, tag="hs")
                nc.vector.memset(hs_t[:, :, 72:128], 0.0)
                for j1 in range(j0, j0 + grp, SUB):
                    sub = min(SUB, j0 + grp - j1)
                    ph = php.tile([128, SUB, 72], F32, tag="ph")
                    for j in range(j1, j1 + sub):
                        nc.tensor.matmul(
                            out=ph[:, j - j1, :],
                            lhsT=xt_t[:, j * 128:(j + 1) * 128],
                            rhs=re_t[:], start=True, stop=True)
                    nc.vector.tensor_copy(
                        out=hs_t[:, j1 - j0:j1 - j0 + sub, 0:72],
                        in_=ph[:, :sub, :])
                    for j in range(j1, j1 + sub):
                        bi = (n_done + j * 128) // 128
                        if bi < WINS:
                            nc.vector.tensor_copy(
                                out=sdall[:, bi, :],
                                in_=ph[:, j - j1, 68:72])
                # swizzled write: partition p -> tab rows p*NBLK + [b0, b0+grp)
                b0 = (n_done + j0 * 128) // 128
                dst_ap = _make_ap(
                    tab_d[:], b0 * 128,
                    [[NBLK * 128, 128], [128, grp], [1, 128]])
                nc.sync.dma_start(out=dst_ap, in_=hs_t[:, :grp, :])
            n_done += csz


def _phase2(nc, tc, pools, Br, nidx, coff, Btot, iota_t, iop_t, sdall,
            idx_d, dstc_d, dstf_d, wc_d, tab_d, out_d, gp_bufs):
    ld, gp, cw, op, pss, psa = pools
    ot = None
    for w in range(WINS):
        idx_t = ld.tile([128, Btot * 8], I16, tag="idx")
        nc.sync.dma_start(out=idx_t[:],
                          in_=idx_d[:, w * Btot * 8:(w + 1) * Btot * 8])
        dstc_t = ld.tile([128, Btot], BF16, tag="dstc")
        nc.sync.dma_start(out=dstc_t[:],
                          in_=dstc_d[:, w * Btot:(w + 1) * Btot])
        wc_t = ld.tile([128, Btot], F32, tag="wc")
        nc.sync.dma_start(out=wc_t[:],
                          in_=wc_d[:, w * Btot:(w + 1) * Btot])
        # partition-broadcast of dstf slice (stride-0 DRAM src)
        bc_t = ld.tile([128, Btot * 128], I8, tag="bc")
        src_ap = bass.AP(dstf_d[:].tensor, w * Btot * 128,
                         [[0, 128], [1, Btot * 128]])
        nc.sync.dma_start(out=bc_t[:], in_=src_ap)

        g = gp.tile([128, Btot, 128], BF16, tag="g")
        ncols = [0] * NCLASS
        for r in range(NCLASS):
            n_wr = nidx[w][r]
            if n_wr:
                ncols[r] = n_wr // 128
                view = tab_d[r * 32768: r * 32768 + CLS_ROWS[r]]
                nc.gpsimd.dma_gather(
                    g[:, coff[r]:coff[r] + ncols[r], :], view,
                    idx_t[:, coff[r] * 8:coff[r] * 8 + n_wr // 16],
                    n_wr, n_wr, 128)
            if coff[r] + ncols[r] < coff[r + 1]:
                # tail columns never gathered: zero them (disjoint from the
                # gather writes, so no WAW serialization on the DMA path)
                nc.vector.memset(g[:, coff[r] + ncols[r]:coff[r + 1], :], 0.0)

        oh = cw.tile([128, Btot, 128], BF16, tag="oh")
        nc.vector.tensor_tensor(
            out=oh[:], in0=_bcast_dim(iota_t[:], 1, Btot),
            in1=dstc_t[:].to_broadcast([128, Btot, 128]),
            op=mybir.AluOpType.is_equal)
        ohT = cw.tile([128, Btot * 128], BF16, tag="ohT")
        nc.vector.tensor_tensor(
            out=ohT[:], in0=bc_t[:],
            in1=iop_t[:].to_broadcast([128, Btot * 128]),
            op=mybir.AluOpType.is_equal)

        sdwb = cw.tile([128, 4], BF16, tag="sdwb")
        nc.vector.tensor_copy(out=sdwb[:], in_=sdall[:, w, :])
        sde_ps = pss.tile([128, Btot * 4], F32, tag="sde")
        for c in range(Btot):
            nc.tensor.matmul(out=sde_ps[:, c * 4:(c + 1) * 4],
                             lhsT=ohT[:, c * 128:(c + 1) * 128],
                             rhs=sdwb[:], start=True, stop=True)

        logit = cw.tile([128, Btot, 4], F32, tag="logit")
        nc.vector.tensor_add(
            out=logit[:], in0=g[:, :, 64:68],
            in1=sde_ps[:].rearrange("p (a b) -> p a b", b=4))
        nc.vector.scalar_tensor_tensor(
            out=logit[:], in0=logit[:], scalar=NEG, in1=logit[:],
            op0=mybir.AluOpType.mult, op1=mybir.AluOpType.max)
        nc.vector.tensor_mul(
            out=logit[:], in0=logit[:],
            in1=wc_t[:].to_broadcast([128, Btot, 4]))
        p = cw.tile([128, Btot, 4], F32, tag="p")
        nc.scalar.activation(p[:], logit[:],
                             mybir.ActivationFunctionType.Exp)

        pay = cw.tile([128, Btot, 68], BF16, tag="pay")
        pv = p[:].to_broadcast([128, Btot, 4, 16])
        gv = g[:, :, 0:64].rearrange("p k (h d) -> p k h d", d=16)
        ov = pay[:, :, 0:64].rearrange("p k (h d) -> p k h d", d=16)
        nc.vector.tensor_mul(out=ov, in0=gv, in1=pv)
        nc.vector.tensor_copy(out=pay[:, :, 64:68], in_=p[:])

        used = [coff[r] + j for r in range(NCLASS) for j in range(ncols[r])]
        acc = psa.tile([128, 68], F32, tag="acc")
        for i, c in enumerate(used):
            nc.tensor.matmul(
                out=acc[:], lhsT=oh[:, c, :], rhs=pay[:, c, :],
                start=(i == 0), stop=(i == len(used) - 1))

        if w % OG == 0:
            ot = op.tile([128, OG, 64], F32, tag="ot")
        den = cw.tile([128, 4], F32, tag="den")
        nc.vector.tensor_scalar_add(out=den[:], in0=acc[:, 64:68],
                                    scalar1=EPS)
        rec = cw.tile([128, 4], F32, tag="rec")
        nc.vector.reciprocal(out=rec[:], in_=den[:])
        nc.vector.tensor_mul(
            out=ot[:, w % OG, :].rearrange("p (h d) -> p h d", d=16),
            in0=acc[:, 0:64].rearrange("p (h d) -> p h d", d=16),
            in1=rec[:].to_broadcast([128, 4, 16]))
        if w % OG == OG - 1:
            r0 = (w - OG + 1) * 128
            dst_ap = _make_ap(out_d[:], r0 * 64,
                              [[64, 128], [128 * 64, OG], [1, 64]])
            nc.sync.dma_start(out=dst_ap, in_=ot[:])


def _build_program(meta, num_devices=CORES, repeat=1):
    Br, nidx = meta
    coff = [0]
    for b in Br:
        coff.append(coff[-1] + b)
    Btot = coff[-1]

    nc = bacc.Bacc("TRN2", target_bir_lowering=False, debug=False,
                   enable_asserts=False, num_devices=num_devices)
    xT_d = nc.dram_tensor("xT", [IN_CH, NP_PAD], BF16, kind="ExternalInput")
    re_d = nc.dram_tensor("rhs_ext", [IN_CH, 72], BF16, kind="ExternalInput")
    io_d = nc.dram_tensor("iota", [128, 128], BF16, kind="ExternalInput")
    iop_d = nc.dram_tensor("iop", [128, 1], I8, kind="ExternalInput")
    idx_d = nc.dram_tensor("idx16", [128, WINS * Btot * 8], I16,
                           kind="ExternalInput")
    dstc_d = nc.dram_tensor("dstc", [128, WINS * Btot], BF16,
                            kind="ExternalInput")
    dstf_d = nc.dram_tensor("dstf", [1, WINS * Btot * 128], I8,
                            kind="ExternalInput")
    wc_d = nc.dram_tensor("wc", [128, WINS * Btot], F32,
                          kind="ExternalInput")
    tab_d = nc.dram_tensor("tab", [NP_PAD, 128], BF16, kind="Internal")
    out_d = nc.dram_tensor("out", [OWN_PAD, 64], F32, kind="ExternalOutput")

    with tile.TileContext(nc) as tc, ExitStack() as ctx:
        const = ctx.enter_context(tc.tile_pool(name="const", bufs=1))
        iota_t = const.tile([128, 128], BF16)
        nc.sync.dma_start(out=iota_t[:], in_=io_d[:])
        iop_t = const.tile([128, 1], I8)
        nc.sync.dma_start(out=iop_t[:], in_=iop_d[:])
        re_t = const.tile([128, 72], BF16)
        nc.sync.dma_start(out=re_t[:], in_=re_d[:])
        sdp = ctx.enter_context(tc.tile_pool(name="sdp", bufs=1))

        ld = ctx.enter_context(tc.tile_pool(name="ld", bufs=6))
        gp = ctx.enter_context(tc.tile_pool(name="gp", bufs=4))
        cw = ctx.enter_context(tc.tile_pool(name="cw", bufs=3))
        op = ctx.enter_context(tc.tile_pool(name="op", bufs=2))
        pss = ctx.enter_context(tc.tile_pool(name="pss", bufs=2, space="PSUM"))
        psa = ctx.enter_context(tc.tile_pool(name="psa", bufs=2, space="PSUM"))
        pools = (ld, gp, cw, op, pss, psa)

        for _rep in range(repeat):
            sdall = sdp.tile([128, WINS, 4], F32, tag="sdall")
            _phase1(nc, tc, xT_d, re_t, tab_d, sdall)
            _phase2(nc, tc, pools, Br, nidx, coff, Btot, iota_t, iop_t,
                    sdall, idx_d, dstc_d, dstf_d, wc_d, tab_d, out_d,
                    4 if _rep == 0 else 0)

    nc.compile()
    return nc


def kernel(x, edge_index, edge_weight, W, a):
    global LAST_EXEC_NS, LAST_NC, LAST_IN_MAPS, LAST_BR
    t0 = time.time()
    in_maps, meta = _host_prep(x, edge_index, edge_weight, W, a)
    Br = meta[0]
    t1 = time.time()
    nc = _build_program(meta)
    LAST_NC = nc
    LAST_IN_MAPS = in_maps
    LAST_BR = meta
    t2 = time.time()
    res = bass_utils.run_bass_kernel_spmd(
        nc, in_maps, core_ids=list(range(CORES)))
    t3 = time.time()
    print(f"[kernel] host_prep {t1-t0:.1f}s  build+compile {t2-t1:.1f}s  "
          f"exec(all-in) {t3-t2:.1f}s  Br={Br}")
    LAST_EXEC_NS = res.exec_time_ns
    parts = [res.results[c]["out"][:NPC] for c in range(CORES)]
    return np.ascontiguousarray(np.concatenate(parts, axis=0))
